# revision 1
# baseline (speedup 1.0000x reference)
"""MoE router gate (group-limited top-k) on 8 Trainium2 NeuronCores.

reference math (per token row of x [T=16384, D=4096], W [E=256, D]):
  logits = x @ W.T                      [T, 256]
  scores = softmax(logits)
  group (8 groups of 32) scores = max of scores per group
  keep top-4 groups, mask rest to -inf
  top-8 experts over masked scores -> indices
  weights = gathered softmax scores, renormalized over the 8 (+1e-9 in denom)

Sharding: data-parallel over tokens, 2048 tokens/core; W.T replicated
(transposed once on host so the contraction dim D lands on SBUF partitions).

Per-core kernel, for each 128-token tile:
  - DMA x block [128, 4096]
  - PE-transpose x chunks [128t,128d] -> [128d,128t] (fp32 transpose mode),
    ACT-copy PSUM->SBUF, fp32 matmul accumulate logits [128t, 256e] in PSUM
  - selection on raw logits (softmax is monotone per row):
      group maxes via 3D tensor_reduce, group top-4 threshold via DVE max
      (sorted top-8), additive -1e30 mask, DVE max + max_index for expert
      top-8 values/indices
  - weights = exp(v - M) / (sum8 + 1e-9 * Z), Z = full-row exp sum, via ACT
    activation(Exp, bias=-M, accum_out)

fp32 PE instructions self-load weights (lower to the S3_LW struct) and only
carry ONE sync-wait slot, so the program keeps every PE instruction at <=1
semaphore wait: sacrificial transposes into a scratch PSUM bank absorb the
identity/wt/x DMA deps, and every PSUM tile freed for PE reuse has exactly
one reader engine (ACT).
"""

import numpy as np

from concourse import bass, mybir
from concourse.bacc import Bacc
from concourse.tile import TileContext
from concourse.bass_utils import run_bass_kernel_spmd

TOKENS = 16384
DIM = 4096
E = 256
TOPK = 8
G = 8
GSZ = E // G  # 32
NL = 4  # groups kept
N_CORES = 8
TPC = TOKENS // N_CORES  # 2048 tokens per core
NT = TPC // 128  # 16 token tiles per core
KC = DIM // 128  # 32 contraction chunks
NEG_BIG = -1.0e30
USE_F32R = False  # f32r = reduced precision (81/16384 idx flips on HW); keep fp32
REPEAT = 1  # bench-only: replicate the tile loop on device

_CACHE = {}
_EYE = np.eye(128, dtype=np.float32)


def _build_program():
    nc = Bacc()
    x_ext = nc.declare_dram_parameter("x", [TPC, DIM], mybir.dt.float32, isOutput=False)
    id_ext = nc.declare_dram_parameter(
        "ident", [128, 128], mybir.dt.float32, isOutput=False
    )
    wt_ext = nc.declare_dram_parameter("wt", [DIM, E], mybir.dt.float32, isOutput=False)
    w_out = nc.declare_dram_parameter(
        "weights", [TPC, TOPK], mybir.dt.float32, isOutput=True
    )
    i_out = nc.declare_dram_parameter(
        "indices", [TPC, TOPK], mybir.dt.int32, isOutput=True
    )

    f32 = mybir.dt.float32

    with TileContext(nc) as tc:
        with (
            tc.tile_pool(name="const", bufs=1) as const_pool,
            tc.tile_pool(name="xb", bufs=3) as xb_pool,
            tc.tile_pool(name="sxt", bufs=4) as sxt_pool,
            tc.tile_pool(name="pxt", bufs=4, space="PSUM") as pxt_pool,
            tc.tile_pool(name="plg", bufs=3, space="PSUM") as plg_pool,
            tc.tile_pool(name="mid", bufs=3) as mid_pool,
            tc.tile_pool(name="small", bufs=3) as small_pool,
        ):
            ident = const_pool.tile([128, 128], f32, tag="ident")
            nc.sync.dma_start(out=ident[:], in_=id_ext[:])

            # W.T resident in SBUF: chunk k occupies columns [k*E, (k+1)*E),
            # partitions = contraction dim d within chunk. Single DMA.
            wt_sb = const_pool.tile([128, KC * E], f32, tag="wt")
            nc.sync.dma_start(
                out=wt_sb[:].rearrange("p (k e) -> p k e", k=KC),
                in_=wt_ext[:].rearrange("(k p) e -> p k e", p=128),
            )

            if USE_F32R:
                # rounding copy: fp32 -> fp32r so the matmul rhs is legal
                wt_r = const_pool.tile([128, KC * E], mybir.dt.float32r, tag="wtr")
                nc.scalar.copy(wt_r[:], wt_sb[:])
                rhs_src = wt_r
            else:
                rhs_src = wt_sb


            for _r in range(REPEAT):
              for t in range(NT):
                xb = xb_pool.tile([128, DIM], f32, tag="xb")
                nc.sync.dma_start(out=xb[:], in_=x_ext[t * 128 : (t + 1) * 128, :])

                lg = plg_pool.tile([128, E], f32, tag="lg")
                for kb in range(KC // 4):
                    pxt = pxt_pool.tile([128, 512], f32, tag="pxt")
                    for j in range(4):
                        k = kb * 4 + j
                        nc.tensor.transpose(
                            pxt[:, j * 128 : (j + 1) * 128],
                            xb[:, k * 128 : (k + 1) * 128],
                            ident[:],
                        )
                    sxt_dt = mybir.dt.float32r if USE_F32R else f32
                    sxt = sxt_pool.tile([128, 512], sxt_dt, tag="sxt")
                    nc.scalar.copy(sxt[:], pxt[:])
                    for j in range(4):
                        k = kb * 4 + j
                        nc.tensor.matmul(
                            lg[:],
                            lhsT=sxt[:, j * 128 : (j + 1) * 128],
                            rhs=rhs_src[:, k * E : (k + 1) * E],
                            start=(k == 0),
                            stop=(k == KC - 1),
                        )

                # single PSUM reader (ACT), so lg's bank frees with one sem
                logits = mid_pool.tile([128, E], f32, tag="logits")
                nc.scalar.copy(logits[:], lg[:])

                # ---- selection on raw logits ----
                gs = small_pool.tile([128, G], f32, tag="gs")
                nc.vector.tensor_reduce(
                    gs[:],
                    logits[:].rearrange("p (g e) -> p g e", g=G),
                    axis=mybir.AxisListType.X,
                    op=mybir.AluOpType.max,
                )
                gsort = small_pool.tile([128, 8], f32, tag="gsort")
                nc.vector.max(out=gsort[:], in_=gs[:])
                # bias per group: (gs < 4th-largest) * -1e30
                bias8 = small_pool.tile([128, G], f32, tag="bias8")
                nc.vector.tensor_scalar(
                    bias8[:],
                    gs[:],
                    gsort[:, NL - 1 : NL],
                    NEG_BIG,
                    op0=mybir.AluOpType.is_lt,
                    op1=mybir.AluOpType.mult,
                )
                masked = mid_pool.tile([128, E], f32, tag="masked")
                for g in range(G):
                    nc.vector.tensor_scalar_add(
                        masked[:, g * GSZ : (g + 1) * GSZ],
                        logits[:, g * GSZ : (g + 1) * GSZ],
                        bias8[:, g : g + 1],
                    )
                vals8 = small_pool.tile([128, 8], f32, tag="vals8")
                nc.vector.max(out=vals8[:], in_=masked[:])
                idx8 = small_pool.tile([128, 8], mybir.dt.uint32, tag="idx8")
                nc.vector.max_index(out=idx8[:], in_max=vals8[:], in_values=masked[:])

                # ---- weights: e_k / (S + 1e-9 * Z), shifted by M = top value
                negm = small_pool.tile([128, 1], f32, tag="negm")
                nc.vector.tensor_scalar_mul(negm[:], vals8[:, 0:1], -1.0)
                scr = mid_pool.tile([128, E], f32, tag="scr")
                zfull = small_pool.tile([128, 1], f32, tag="zfull")
                nc.scalar.activation(
                    scr[:],
                    logits[:],
                    mybir.ActivationFunctionType.Exp,
                    bias=negm[:],
                    accum_out=zfull[:],
                )
                e8 = small_pool.tile([128, 8], f32, tag="e8")
                s8 = small_pool.tile([128, 1], f32, tag="s8")
                nc.scalar.activation(
                    e8[:],
                    vals8[:],
                    mybir.ActivationFunctionType.Exp,
                    bias=negm[:],
                    accum_out=s8[:],
                )
                den = small_pool.tile([128, 1], f32, tag="den")
                nc.vector.tensor_scalar(
                    den[:],
                    zfull[:],
                    1.0e-9,
                    None,
                    op0=mybir.AluOpType.mult,
                )
                nc.vector.tensor_add(den[:], den[:], s8[:])
                rcp = small_pool.tile([128, 1], f32, tag="rcp")
                nc.vector.reciprocal(rcp[:], den[:])
                w8 = small_pool.tile([128, 8], f32, tag="w8")
                nc.vector.tensor_scalar_mul(w8[:], e8[:], rcp[:])
                i32 = small_pool.tile([128, 8], mybir.dt.int32, tag="i32")
                nc.vector.tensor_copy(out=i32[:], in_=idx8[:])

                nc.sync.dma_start(
                    out=w_out[t * 128 : (t + 1) * 128, :], in_=w8[:]
                )
                nc.sync.dma_start(
                    out=i_out[t * 128 : (t + 1) * 128, :], in_=i32[:]
                )
    return nc


def get_program():
    if "nc" not in _CACHE:
        nc = _build_program()
        # Bacc defers register allocation + wait-splitting to finalize();
        # the PJRT path serializes the module as-is, so lower it now.
        nc.finalize()
        _CACHE["nc"] = nc
    return _CACHE["nc"]


def kernel(x: np.ndarray, weight: np.ndarray, **run_kwargs):
    x = np.ascontiguousarray(x, dtype=np.float32)
    wt = np.ascontiguousarray(weight.T, dtype=np.float32)  # [DIM, E]
    nc = get_program()
    in_maps = [
        {"x": x[c * TPC : (c + 1) * TPC], "wt": wt, "ident": _EYE}
        for c in range(N_CORES)
    ]
    res = run_bass_kernel_spmd(nc, in_maps, list(range(N_CORES)), **run_kwargs)
    weights = np.concatenate([res.results[c]["weights"] for c in range(N_CORES)], axis=0)
    indices = np.concatenate([res.results[c]["indices"] for c in range(N_CORES)], axis=0)
    _CACHE["last_results"] = res
    return weights.astype(np.float32), indices.astype(np.int32)

